# revision 2
# baseline (speedup 1.0000x reference)
"""DeepseekV3 MoE MLP (grouped ragged GEMM) on 8 Trainium2 NeuronCores.

Strategy: expert-parallel. 32 experts / 8 cores = 4 experts per core; each
core processes its experts' token groups (tokens arrive pre-sorted by
expert). Compute in bf16 (fp32 accumulation in PSUM), fp32 output.

Per-core pipeline, per expert (H=2048, I=1408, C tokens padded):
  stage 1:  gateT[i,t] = sum_h W1[h,i] * XT[h,t]   (W1 tile = lhsT, XT = rhs)
            upT  [i,t] = sum_h W2[h,i] * XT[h,t]
            h2T  [i,t] = silu(gateT) * upT          (ScalarE Silu + VectorE mul)
  stage 2:  down [t,h] = sum_i h2T[i,t] * W3[i,h]   (h2T tile = lhsT, W3 = rhs)

All operands are laid out host-side so every DMA is 128 partitions x
contiguous per-partition blocks; no on-device transposes anywhere.
"""

import numpy as np
import ml_dtypes

# Problem constants (hardcoded per contract).
E = 32          # experts
H = 2048        # hidden dim
I = 1408        # moe intermediate dim
N_CORES = 8
EPC = E // N_CORES  # experts per core
P = 128
HO = H // P     # 16 h-subtiles
IT = I // P     # 11 i-subtiles
HC = H // 512   # 4 output h-chunks of 512

BF16 = ml_dtypes.bfloat16

_PROGRAM_CACHE: dict = {}


def _build_program(C: int):
    """Build + compile the per-core Bass program for per-expert capacity C
    (multiple of 512). Returns (nc, meta)."""
    import concourse.bacc as bacc
    import concourse.mybir as mybir
    import concourse.tile as tile

    NT = C // 512   # stage-1 token chunks of 512
    TT = C // P     # stage-2 token tiles of 128

    nc = bacc.Bacc("TRN2", debug=False, num_devices=N_CORES)

    xt = nc.dram_tensor("xt", [EPC * NT, P, HO, 512], mybir.dt.bfloat16,
                        kind="ExternalInput").ap()
    w1 = nc.dram_tensor("w1", [EPC, IT, P, HO, P], mybir.dt.bfloat16,
                        kind="ExternalInput").ap()
    w2 = nc.dram_tensor("w2", [EPC, IT, P, HO, P], mybir.dt.bfloat16,
                        kind="ExternalInput").ap()
    w3 = nc.dram_tensor("w3", [EPC, HC, P, IT, 512], mybir.dt.bfloat16,
                        kind="ExternalInput").ap()
    out = nc.dram_tensor("out", [EPC * C, H], mybir.dt.float32,
                         kind="ExternalOutput").ap()

    with tile.TileContext(nc) as tc:
        with (
            tc.tile_pool(name="xt_pool", bufs=2) as xt_pool,
            tc.tile_pool(name="w12_pool", bufs=3) as w12_pool,
            tc.tile_pool(name="w3_pool", bufs=2) as w3_pool,
            tc.tile_pool(name="h2t_pool", bufs=2) as h2t_pool,
            tc.tile_pool(name="act_pool", bufs=3) as act_pool,
            tc.tile_pool(name="out_pool", bufs=4) as out_pool,
            tc.tile_pool(name="ps_g", bufs=2, space="PSUM") as ps_g,
            tc.tile_pool(name="ps_u", bufs=2, space="PSUM") as ps_u,
            tc.tile_pool(name="ps_d", bufs=3, space="PSUM") as ps_d,
        ):
            for e in range(EPC):
                # ---- load this expert's XT token chunks ----
                xt_tiles = []
                for tch in range(NT):
                    t_sb = xt_pool.tile([P, HO, 512], mybir.dt.bfloat16, tag="xt")
                    nc.sync.dma_start(out=t_sb[:], in_=xt[e * NT + tch])
                    xt_tiles.append(t_sb)

                h2t = h2t_pool.tile([P, IT, C], mybir.dt.bfloat16, tag="h2t")

                # ---- stage 1: gateT/upT + silu*mul -> h2T ----
                for it in range(IT):
                    w1_sb = w12_pool.tile([P, HO, P], mybir.dt.bfloat16, tag="w1")
                    nc.sync.dma_start(out=w1_sb[:], in_=w1[e, it])
                    w2_sb = w12_pool.tile([P, HO, P], mybir.dt.bfloat16, tag="w2")
                    nc.sync.dma_start(out=w2_sb[:], in_=w2[e, it])

                    for tch in range(NT):
                        pg = ps_g.tile([P, 512], mybir.dt.float32, tag="pg")
                        pu = ps_u.tile([P, 512], mybir.dt.float32, tag="pu")
                        for ho in range(HO):
                            nc.tensor.matmul(pg, w1_sb[:, ho], xt_tiles[tch][:, ho],
                                             start=(ho == 0), stop=(ho == HO - 1))
                        for ho in range(HO):
                            nc.tensor.matmul(pu, w2_sb[:, ho], xt_tiles[tch][:, ho],
                                             start=(ho == 0), stop=(ho == HO - 1))
                        sil = act_pool.tile([P, 512], mybir.dt.float32, tag="sil")
                        nc.scalar.activation(sil, pg,
                                             mybir.ActivationFunctionType.Silu)
                        nc.vector.tensor_mul(
                            h2t[:, it, tch * 512:(tch + 1) * 512], sil, pu)

                # ---- stage 2: down = h2 @ W3 ----
                for hc in range(HC):
                    w3_sb = w3_pool.tile([P, IT, 512], mybir.dt.bfloat16, tag="w3")
                    nc.sync.dma_start(out=w3_sb[:], in_=w3[e, hc])
                    for tt in range(TT):
                        pd = ps_d.tile([P, 512], mybir.dt.float32, tag="pd")
                        for io in range(IT):
                            nc.tensor.matmul(
                                pd, h2t[:, io, tt * P:(tt + 1) * P], w3_sb[:, io],
                                start=(io == 0), stop=(io == IT - 1))
                        ot = out_pool.tile([P, 512], mybir.dt.float32, tag="ot")
                        nc.scalar.copy(ot, pd)
                        nc.sync.dma_start(
                            out=out[e * C + tt * P: e * C + (tt + 1) * P,
                                    hc * 512:(hc + 1) * 512],
                            in_=ot[:])

    nc.compile()
    return nc


def _prep_inputs(hidden_states, gate_w, up_w, down_w, group_sizes, C):
    """Host-side: group tokens by expert (padded to C), transpose, convert to
    bf16, and pre-tile everything into the DMA layouts the program expects.
    Returns (in_maps, offsets)."""
    T = hidden_states.shape[0]
    gs = np.asarray(group_sizes, dtype=np.int64)
    offsets = np.zeros(E + 1, dtype=np.int64)
    np.cumsum(gs, out=offsets[1:])
    assert offsets[-1] == T, f"group_sizes sum {offsets[-1]} != T {T}"

    # Pad each expert's token block to C rows, convert to bf16.
    x_pad = np.zeros((E, C, H), dtype=BF16)
    for e in range(E):
        x_pad[e, :gs[e]] = hidden_states[offsets[e]:offsets[e + 1]]

    NT = C // 512
    # XT layout: [core][e_local*NT + tch][p][ho][512] with h = ho*128 + p
    # x_pad: [E, C, H] -> [E, NT, 512, HO, P] -> transpose to [E, NT, P, HO, 512]
    xt_all = np.ascontiguousarray(
        x_pad.reshape(E, NT, 512, HO, P).transpose(0, 1, 4, 3, 2)
    ).reshape(N_CORES, EPC * NT, P, HO, 512)

    # W1/W2 layout: [E][it][p][ho][128i] with h = ho*128 + p
    def tile_w12(w):
        wb = np.asarray(w, dtype=BF16)
        return np.ascontiguousarray(
            wb.reshape(E, HO, P, IT, P).transpose(0, 3, 2, 1, 4)
        ).reshape(N_CORES, EPC, IT, P, HO, P)

    w1_all = tile_w12(gate_w)
    w2_all = tile_w12(up_w)

    # W3 layout: [E][hc][p][io][512h] with i = io*128 + p
    w3b = np.asarray(down_w, dtype=BF16)
    w3_all = np.ascontiguousarray(
        w3b.reshape(E, IT, P, HC, 512).transpose(0, 3, 2, 1, 4)
    ).reshape(N_CORES, EPC, HC, P, IT, 512)

    in_maps = [
        {"xt": xt_all[c], "w1": w1_all[c], "w2": w2_all[c], "w3": w3_all[c]}
        for c in range(N_CORES)
    ]
    return in_maps, offsets, gs


def _run(hidden_states, gate_w, up_w, down_w, group_sizes, trace=False):
    from concourse.bass_utils import run_bass_kernel_spmd

    gs = np.asarray(group_sizes, dtype=np.int64)
    max_g = int(gs.max()) if gs.size else 512
    C = max(512, -(-max_g // 512) * 512)  # round up to multiple of 512

    key = ("v1", C)
    if key not in _PROGRAM_CACHE:
        _PROGRAM_CACHE[key] = _build_program(C)
    nc = _PROGRAM_CACHE[key]

    in_maps, offsets, gs = _prep_inputs(
        hidden_states, gate_w, up_w, down_w, group_sizes, C)

    res = run_bass_kernel_spmd(nc, in_maps, core_ids=list(range(N_CORES)),
                               trace=trace)
    global LAST_RES
    LAST_RES = res

    T = hidden_states.shape[0]
    out_full = np.empty((T, H), dtype=np.float32)
    for c in range(N_CORES):
        core_out = res.results[c]["out"]  # [EPC*C, H] fp32
        for el in range(EPC):
            e = c * EPC + el
            out_full[offsets[e]:offsets[e + 1]] = \
                core_out[el * C: el * C + gs[e]]
    return out_full, res.exec_time_ns


def kernel(hidden_states, gate_w, up_w, down_w, group_sizes):
    out, _ = _run(hidden_states, gate_w, up_w, down_w, group_sizes)
    return out



# revision 6
# speedup vs baseline: 1.0043x; 1.0043x over previous
"""DeepseekV3 MoE MLP (grouped ragged GEMM) on 8 Trainium2 NeuronCores.

Strategy: expert-parallel. 32 experts / 8 cores = 4 experts per core; each
core processes its experts' token groups (tokens arrive pre-sorted by
expert). Compute in bf16 (fp32 accumulation in PSUM), fp32 output.

Per-core pipeline, per expert (H=2048, I=1408, C tokens padded):
  stage 1:  gateT[i,t] = sum_h W1[h,i] * XT[h,t]   (W1 tile = lhsT, XT = rhs)
            upT  [i,t] = sum_h W2[h,i] * XT[h,t]
            h2T  [i,t] = silu(gateT) * upT          (ScalarE Silu + VectorE mul)
  stage 2:  down [t,h] = sum_i h2T[i,t] * W3[i,h]   (h2T tile = lhsT, W3 = rhs)

All operands are laid out host-side so every DMA is 128 partitions x
contiguous per-partition blocks; no on-device transposes anywhere.
"""

import numpy as np
import ml_dtypes

# Problem constants (hardcoded per contract).
E = 32          # experts
H = 2048        # hidden dim
I = 1408        # moe intermediate dim
N_CORES = 8
EPC = E // N_CORES  # experts per core
P = 128
HO = H // P     # 16 h-subtiles
IT = I // P     # 11 i-subtiles
HC = H // 512   # 4 output h-chunks of 512

BF16 = ml_dtypes.bfloat16

_PROGRAM_CACHE: dict = {}


def _build_program(C: int):
    """Build + compile the per-core Bass program for per-expert capacity C
    (multiple of 512). Returns (nc, meta)."""
    import concourse.bacc as bacc
    import concourse.mybir as mybir
    import concourse.tile as tile

    NT = C // 512   # stage-1 token chunks of 512
    TT = C // P     # stage-2 token tiles of 128

    nc = bacc.Bacc("TRN2", debug=False, num_devices=N_CORES)

    xt = nc.dram_tensor("xt", [EPC * NT, P, HO, 512], mybir.dt.bfloat16,
                        kind="ExternalInput").ap()
    w1 = nc.dram_tensor("w1", [EPC, IT, P, HO, P], mybir.dt.bfloat16,
                        kind="ExternalInput").ap()
    w2 = nc.dram_tensor("w2", [EPC, IT, P, HO, P], mybir.dt.bfloat16,
                        kind="ExternalInput").ap()
    w3 = nc.dram_tensor("w3", [EPC, HC, P, IT, 512], mybir.dt.bfloat16,
                        kind="ExternalInput").ap()
    out = nc.dram_tensor("out", [EPC * C, H], mybir.dt.float32,
                         kind="ExternalOutput").ap()

    with tile.TileContext(nc) as tc:
        with (
            tc.tile_pool(name="xt_pool", bufs=2) as xt_pool,
            tc.tile_pool(name="w12_pool", bufs=3) as w12_pool,
            tc.tile_pool(name="w3_pool", bufs=2) as w3_pool,
            tc.tile_pool(name="h2t_pool", bufs=2) as h2t_pool,
            tc.tile_pool(name="act_pool", bufs=3) as act_pool,
            tc.tile_pool(name="warm_pool", bufs=1) as warm_pool,
            tc.tile_pool(name="out_pool", bufs=4) as out_pool,
            tc.tile_pool(name="ps_g", bufs=2, space="PSUM") as ps_g,
            tc.tile_pool(name="ps_u", bufs=2, space="PSUM") as ps_u,
            tc.tile_pool(name="ps_d", bufs=4, space="PSUM") as ps_d,
        ):
            # ---- PE warm-up: dummy matmuls on a zeroed tile keep TensorE
            # busy through the initial DMA wait so the HAM clock gate is at
            # 8/8 (2.4 GHz) when the first real matmuls issue. Results land
            # in a scratch PSUM bank and are never read. ----
            warm = warm_pool.tile([P, 512], mybir.dt.bfloat16, tag="warm")
            nc.vector.memset(warm, 0.0)
            ps_w = ps_d.tile([P, 512], mybir.dt.float32, tag="pd")
            for i in range(14):
                nc.tensor.matmul(ps_w, warm[:, 0:P], warm,
                                 start=(i == 0), stop=False)
            for i in range(16):
                nc.tensor.matmul(ps_w[:, 0:P], warm[:, 0:P], warm[:, 0:P],
                                 start=False, stop=(i == 15))

            for e in range(EPC):
                # ---- load this expert's XT token chunks ----
                xt_tiles = []
                for tch in range(NT):
                    t_sb = xt_pool.tile([P, HO, 512], mybir.dt.bfloat16, tag="xt")
                    nc.sync.dma_start(out=t_sb[:], in_=xt[e * NT + tch])
                    xt_tiles.append(t_sb)

                h2t = h2t_pool.tile([P, IT, C], mybir.dt.bfloat16, tag="h2t")

                # ---- stage 1: gateT/upT + silu*mul -> h2T ----
                for it in range(IT):
                    w1_sb = w12_pool.tile([P, HO, P], mybir.dt.bfloat16, tag="w1")
                    nc.sync.dma_start(out=w1_sb[:], in_=w1[e, it])
                    w2_sb = w12_pool.tile([P, HO, P], mybir.dt.bfloat16, tag="w2")
                    if e == 0 and it == 0:
                        # Startup-DMA fence: a 1-element write into w2_sb that
                        # reads the xt tile makes the w2 DMA (and, since the
                        # Sync queue is in-order, every later DMA issue) wait
                        # until xt has landed. This keeps the first matmul's
                        # critical 2.6 MB from sharing DMA bandwidth with
                        # lookahead weight loads.
                        nc.vector.tensor_copy(w2_sb[0:1, 0, 0:1],
                                              xt_tiles[0][0:1, 0, 0:1])
                    nc.sync.dma_start(out=w2_sb[:], in_=w2[e, it])

                    for tch in range(NT):
                        pg = ps_g.tile([P, 512], mybir.dt.float32, tag="pg")
                        pu = ps_u.tile([P, 512], mybir.dt.float32, tag="pu")
                        for ho in range(HO):
                            nc.tensor.matmul(pg, w1_sb[:, ho], xt_tiles[tch][:, ho],
                                             start=(ho == 0), stop=(ho == HO - 1))
                        for ho in range(HO):
                            nc.tensor.matmul(pu, w2_sb[:, ho], xt_tiles[tch][:, ho],
                                             start=(ho == 0), stop=(ho == HO - 1))
                        sil = act_pool.tile([P, 512], mybir.dt.float32, tag="sil")
                        nc.scalar.activation(sil, pg,
                                             mybir.ActivationFunctionType.Silu)
                        nc.vector.tensor_mul(
                            h2t[:, it, tch * 512:(tch + 1) * 512], sil, pu)

                # ---- stage 2: down = h2 @ W3 ----
                for hc in range(HC):
                    w3_sb = w3_pool.tile([P, IT, 512], mybir.dt.bfloat16, tag="w3")
                    nc.sync.dma_start(out=w3_sb[:], in_=w3[e, hc])
                    for tt in range(TT):
                        pd = ps_d.tile([P, 512], mybir.dt.float32, tag="pd")
                        for io in range(IT):
                            nc.tensor.matmul(
                                pd, h2t[:, io, tt * P:(tt + 1) * P], w3_sb[:, io],
                                start=(io == 0), stop=(io == IT - 1))
                        rows = slice(e * C + tt * P, e * C + (tt + 1) * P)
                        if e == EPC - 1 and hc == HC - 1 and tt == TT - 1:
                            # Pipeline the very last output tile in quarters
                            # so its copy+DMA tail is short.
                            for q in range(4):
                                oq = out_pool.tile([P, 128], mybir.dt.float32,
                                                   tag="oq")
                                nc.vector.tensor_copy(
                                    oq, pd[:, q * 128:(q + 1) * 128])
                                nc.sync.dma_start(
                                    out=out[rows, hc * 512 + q * 128:
                                            hc * 512 + (q + 1) * 128],
                                    in_=oq[:])
                        else:
                            ot = out_pool.tile([P, 512], mybir.dt.float32,
                                               tag="ot")
                            nc.vector.tensor_copy(ot, pd)
                            nc.sync.dma_start(
                                out=out[rows, hc * 512:(hc + 1) * 512],
                                in_=ot[:])

    nc.compile()
    return nc


def _prep_inputs(hidden_states, gate_w, up_w, down_w, group_sizes, C):
    """Host-side: group tokens by expert (padded to C), transpose, convert to
    bf16, and pre-tile everything into the DMA layouts the program expects.
    Returns (in_maps, offsets)."""
    T = hidden_states.shape[0]
    gs = np.asarray(group_sizes, dtype=np.int64)
    offsets = np.zeros(E + 1, dtype=np.int64)
    np.cumsum(gs, out=offsets[1:])
    assert offsets[-1] == T, f"group_sizes sum {offsets[-1]} != T {T}"

    # Pad each expert's token block to C rows, convert to bf16.
    x_pad = np.zeros((E, C, H), dtype=BF16)
    for e in range(E):
        x_pad[e, :gs[e]] = hidden_states[offsets[e]:offsets[e + 1]]

    NT = C // 512
    # XT layout: [core][e_local*NT + tch][p][ho][512] with h = ho*128 + p
    # x_pad: [E, C, H] -> [E, NT, 512, HO, P] -> transpose to [E, NT, P, HO, 512]
    xt_all = np.ascontiguousarray(
        x_pad.reshape(E, NT, 512, HO, P).transpose(0, 1, 4, 3, 2)
    ).reshape(N_CORES, EPC * NT, P, HO, 512)

    # W1/W2 layout: [E][it][p][ho][128i] with h = ho*128 + p
    def tile_w12(w):
        wb = np.asarray(w, dtype=BF16)
        return np.ascontiguousarray(
            wb.reshape(E, HO, P, IT, P).transpose(0, 3, 2, 1, 4)
        ).reshape(N_CORES, EPC, IT, P, HO, P)

    w1_all = tile_w12(gate_w)
    w2_all = tile_w12(up_w)

    # W3 layout: [E][hc][p][io][512h] with i = io*128 + p
    w3b = np.asarray(down_w, dtype=BF16)
    w3_all = np.ascontiguousarray(
        w3b.reshape(E, IT, P, HC, 512).transpose(0, 3, 2, 1, 4)
    ).reshape(N_CORES, EPC, HC, P, IT, 512)

    in_maps = [
        {"xt": xt_all[c], "w1": w1_all[c], "w2": w2_all[c], "w3": w3_all[c]}
        for c in range(N_CORES)
    ]
    return in_maps, offsets, gs


def _run(hidden_states, gate_w, up_w, down_w, group_sizes, trace=False):
    from concourse.bass_utils import run_bass_kernel_spmd

    gs = np.asarray(group_sizes, dtype=np.int64)
    max_g = int(gs.max()) if gs.size else 512
    C = max(512, -(-max_g // 512) * 512)  # round up to multiple of 512

    key = ("v2", C)
    if key not in _PROGRAM_CACHE:
        _PROGRAM_CACHE[key] = _build_program(C)
    nc = _PROGRAM_CACHE[key]

    in_maps, offsets, gs = _prep_inputs(
        hidden_states, gate_w, up_w, down_w, group_sizes, C)

    res = run_bass_kernel_spmd(nc, in_maps, core_ids=list(range(N_CORES)),
                               trace=trace)
    global LAST_RES
    LAST_RES = res

    T = hidden_states.shape[0]
    out_full = np.empty((T, H), dtype=np.float32)
    for c in range(N_CORES):
        core_out = res.results[c]["out"]  # [EPC*C, H] fp32
        for el in range(EPC):
            e = c * EPC + el
            out_full[offsets[e]:offsets[e + 1]] = \
                core_out[el * C: el * C + gs[e]]
    return out_full, res.exec_time_ns


def kernel(hidden_states, gate_w, up_w, down_w, group_sizes):
    out, _ = _run(hidden_states, gate_w, up_w, down_w, group_sizes)
    return out



# revision 11
# speedup vs baseline: 1.0168x; 1.0124x over previous
"""DeepseekV3 MoE MLP (grouped ragged GEMM) on 8 Trainium2 NeuronCores.

Strategy: expert-parallel. 32 experts / 8 cores = 4 experts per core; each
core processes its experts' token groups (tokens arrive pre-sorted by
expert). Compute in bf16 (fp32 accumulation in PSUM), fp32 output.

Per-core pipeline, per expert (H=2048, I=1408, C tokens padded):
  stage 1:  gateT[i,t] = sum_h W1[h,i] * XT[h,t]   (W1 tile = lhsT, XT = rhs)
            upT  [i,t] = sum_h W2[h,i] * XT[h,t]
            h2T  [i,t] = silu(gateT) * upT          (ScalarE Silu + VectorE mul)
  stage 2:  down [t,h] = sum_i h2T[i,t] * W3[i,h]   (h2T tile = lhsT, W3 = rhs)

All operands are laid out host-side so every DMA is 128 partitions x
contiguous per-partition blocks; no on-device transposes anywhere.
"""

import numpy as np
import ml_dtypes

# Problem constants (hardcoded per contract).
E = 32          # experts
H = 2048        # hidden dim
I = 1408        # moe intermediate dim
N_CORES = 8
EPC = E // N_CORES  # experts per core
P = 128
HO = H // P     # 16 h-subtiles
IT = I // P     # 11 i-subtiles
HC = H // 512   # 4 output h-chunks of 512

BF16 = ml_dtypes.bfloat16

_PROGRAM_CACHE: dict = {}


def _build_program(C: int):
    """Build + compile the per-core Bass program for per-expert capacity C
    (multiple of 512). Returns (nc, meta)."""
    import concourse.bacc as bacc
    import concourse.mybir as mybir
    import concourse.tile as tile

    NT = C // 512   # stage-1 token chunks of 512
    TT = C // P     # stage-2 token tiles of 128

    nc = bacc.Bacc("TRN2", debug=False, num_devices=N_CORES)

    xt = nc.dram_tensor("xt", [EPC * NT, P, HO, 512], mybir.dt.bfloat16,
                        kind="ExternalInput").ap()
    w1 = nc.dram_tensor("w1", [EPC, IT, P, HO, P], mybir.dt.bfloat16,
                        kind="ExternalInput").ap()
    w2 = nc.dram_tensor("w2", [EPC, IT, P, HO, P], mybir.dt.bfloat16,
                        kind="ExternalInput").ap()
    w3 = nc.dram_tensor("w3", [EPC, HC, P, IT, 512], mybir.dt.bfloat16,
                        kind="ExternalInput").ap()
    # Output stored as bf16 (upcast to fp32 host-side): halves store DMA
    # traffic; adds ~0.2% rel err against a 2e-2 budget.
    out = nc.dram_tensor("out", [EPC * C, H], mybir.dt.bfloat16,
                         kind="ExternalOutput").ap()

    with tile.TileContext(nc) as tc:
        with (
            tc.tile_pool(name="xt_pool", bufs=2) as xt_pool,
            tc.tile_pool(name="w12_pool", bufs=3) as w12_pool,
            tc.tile_pool(name="w3_pool", bufs=2) as w3_pool,
            tc.tile_pool(name="h2t_pool", bufs=2) as h2t_pool,
            tc.tile_pool(name="act_pool", bufs=3) as act_pool,
            tc.tile_pool(name="warm_pool", bufs=1) as warm_pool,
            tc.tile_pool(name="out_pool", bufs=4) as out_pool,
            tc.tile_pool(name="ps_g", bufs=2, space="PSUM") as ps_g,
            tc.tile_pool(name="ps_u", bufs=2, space="PSUM") as ps_u,
            tc.tile_pool(name="ps_d", bufs=4, space="PSUM") as ps_d,
        ):
            # ---- PE warm-up: dummy matmuls on a zeroed tile keep TensorE
            # busy through the initial DMA wait so the HAM clock gate is at
            # 8/8 (2.4 GHz) when the first real matmuls issue. Results land
            # in a scratch PSUM bank and are never read. ----
            warm = warm_pool.tile([P, 512], mybir.dt.bfloat16, tag="warm")
            nc.vector.memset(warm, 0.0)
            ps_w = ps_d.tile([P, 512], mybir.dt.float32, tag="pd")
            for i in range(14):
                nc.tensor.matmul(ps_w, warm[:, 0:P], warm,
                                 start=(i == 0), stop=False)
            for i in range(28):
                nc.tensor.matmul(ps_w[:, 0:P], warm[:, 0:P], warm[:, 0:P],
                                 start=False, stop=(i == 27))

            for e in range(EPC):
                # ---- load this expert's XT token chunks ----
                xt_tiles = []
                for tch in range(NT):
                    t_sb = xt_pool.tile([P, HO, 512], mybir.dt.bfloat16, tag="xt")
                    nc.sync.dma_start(out=t_sb[:], in_=xt[e * NT + tch])
                    xt_tiles.append(t_sb)

                h2t = h2t_pool.tile([P, IT, C], mybir.dt.bfloat16, tag="h2t")

                # ---- stage 1: gateT/upT + silu*mul -> h2T ----
                for it in range(IT):
                    w1_sb = w12_pool.tile([P, HO, P], mybir.dt.bfloat16, tag="w1")
                    nc.sync.dma_start(out=w1_sb[:], in_=w1[e, it])
                    w2_sb = w12_pool.tile([P, HO, P], mybir.dt.bfloat16, tag="w2")
                    if e == 0 and it == 0:
                        # Startup-DMA fence: a 1-element write into w2_sb that
                        # reads the xt tile makes the w2 DMA (and, since the
                        # Sync queue is in-order, every later DMA issue) wait
                        # until xt has landed. This keeps the first matmul's
                        # critical 2.6 MB from sharing DMA bandwidth with
                        # lookahead weight loads.
                        nc.vector.tensor_copy(w2_sb[0:1, 0, 0:1],
                                              xt_tiles[0][0:1, 0, 0:1])
                    nc.sync.dma_start(out=w2_sb[:], in_=w2[e, it])

                    for tch in range(NT):
                        pg = ps_g.tile([P, 512], mybir.dt.float32, tag="pg")
                        pu = ps_u.tile([P, 512], mybir.dt.float32, tag="pu")
                        for ho in range(HO):
                            nc.tensor.matmul(pg, w1_sb[:, ho], xt_tiles[tch][:, ho],
                                             start=(ho == 0), stop=(ho == HO - 1))
                        for ho in range(HO):
                            nc.tensor.matmul(pu, w2_sb[:, ho], xt_tiles[tch][:, ho],
                                             start=(ho == 0), stop=(ho == HO - 1))
                        sil = act_pool.tile([P, 512], mybir.dt.float32, tag="sil")
                        nc.scalar.activation(sil, pg,
                                             mybir.ActivationFunctionType.Silu)
                        nc.vector.tensor_mul(
                            h2t[:, it, tch * 512:(tch + 1) * 512], sil, pu)

                # ---- stage 2: down = h2 @ W3 ----
                for hc in range(HC):
                    w3_sb = w3_pool.tile([P, IT, 512], mybir.dt.bfloat16, tag="w3")
                    nc.sync.dma_start(out=w3_sb[:], in_=w3[e, hc])
                    for tt in range(TT):
                        pd = ps_d.tile([P, 512], mybir.dt.float32, tag="pd")
                        for io in range(IT):
                            nc.tensor.matmul(
                                pd, h2t[:, io, tt * P:(tt + 1) * P], w3_sb[:, io],
                                start=(io == 0), stop=(io == IT - 1))
                        rows = slice(e * C + tt * P, e * C + (tt + 1) * P)
                        if e == EPC - 1 and hc == HC - 1 and tt == TT - 1:
                            # Pipeline the very last output tile in halves,
                            # one per HWDGE ring, so its copy+DMA tail is
                            # short.
                            for q, eng in ((0, nc.sync), (1, nc.scalar)):
                                oq = out_pool.tile([P, 256], mybir.dt.bfloat16,
                                                   tag="oq")
                                nc.vector.tensor_copy(
                                    oq, pd[:, q * 256:(q + 1) * 256])
                                eng.dma_start(
                                    out=out[rows, hc * 512 + q * 256:
                                            hc * 512 + (q + 1) * 256],
                                    in_=oq[:])
                        else:
                            ot = out_pool.tile([P, 512], mybir.dt.bfloat16,
                                               tag="ot")
                            nc.vector.tensor_copy(ot, pd)
                            # Output stores go out on the ScalarE HWDGE ring:
                            # they wait on their PSUM->SBUF copies, and on the
                            # in-order Sync ring that wait would head-of-line
                            # block the next expert's weight prefetches.
                            nc.scalar.dma_start(
                                out=out[rows, hc * 512:(hc + 1) * 512],
                                in_=ot[:])

    nc.compile()
    return nc


def _prep_inputs(hidden_states, gate_w, up_w, down_w, group_sizes, C):
    """Host-side: group tokens by expert (padded to C), transpose, convert to
    bf16, and pre-tile everything into the DMA layouts the program expects.
    Returns (in_maps, offsets)."""
    T = hidden_states.shape[0]
    gs = np.asarray(group_sizes, dtype=np.int64)
    offsets = np.zeros(E + 1, dtype=np.int64)
    np.cumsum(gs, out=offsets[1:])
    assert offsets[-1] == T, f"group_sizes sum {offsets[-1]} != T {T}"

    # Pad each expert's token block to C rows, convert to bf16.
    x_pad = np.zeros((E, C, H), dtype=BF16)
    for e in range(E):
        x_pad[e, :gs[e]] = hidden_states[offsets[e]:offsets[e + 1]]

    NT = C // 512
    # XT layout: [core][e_local*NT + tch][p][ho][512] with h = ho*128 + p
    # x_pad: [E, C, H] -> [E, NT, 512, HO, P] -> transpose to [E, NT, P, HO, 512]
    xt_all = np.ascontiguousarray(
        x_pad.reshape(E, NT, 512, HO, P).transpose(0, 1, 4, 3, 2)
    ).reshape(N_CORES, EPC * NT, P, HO, 512)

    # W1/W2 layout: [E][it][p][ho][128i] with h = ho*128 + p
    def tile_w12(w):
        wb = np.asarray(w, dtype=BF16)
        return np.ascontiguousarray(
            wb.reshape(E, HO, P, IT, P).transpose(0, 3, 2, 1, 4)
        ).reshape(N_CORES, EPC, IT, P, HO, P)

    w1_all = tile_w12(gate_w)
    w2_all = tile_w12(up_w)

    # W3 layout: [E][hc][p][io][512h] with i = io*128 + p
    w3b = np.asarray(down_w, dtype=BF16)
    w3_all = np.ascontiguousarray(
        w3b.reshape(E, IT, P, HC, 512).transpose(0, 3, 2, 1, 4)
    ).reshape(N_CORES, EPC, HC, P, IT, 512)

    in_maps = [
        {"xt": xt_all[c], "w1": w1_all[c], "w2": w2_all[c], "w3": w3_all[c]}
        for c in range(N_CORES)
    ]
    return in_maps, offsets, gs


def _run(hidden_states, gate_w, up_w, down_w, group_sizes, trace=False):
    from concourse.bass_utils import run_bass_kernel_spmd

    gs = np.asarray(group_sizes, dtype=np.int64)
    max_g = int(gs.max()) if gs.size else 512
    C = max(512, -(-max_g // 512) * 512)  # round up to multiple of 512

    key = ("v3", C)
    if key not in _PROGRAM_CACHE:
        _PROGRAM_CACHE[key] = _build_program(C)
    nc = _PROGRAM_CACHE[key]

    in_maps, offsets, gs = _prep_inputs(
        hidden_states, gate_w, up_w, down_w, group_sizes, C)

    res = run_bass_kernel_spmd(nc, in_maps, core_ids=list(range(N_CORES)),
                               trace=trace)
    global LAST_RES
    LAST_RES = res

    T = hidden_states.shape[0]
    out_full = np.empty((T, H), dtype=np.float32)
    for c in range(N_CORES):
        core_out = res.results[c]["out"]  # [EPC*C, H] bf16
        for el in range(EPC):
            e = c * EPC + el
            out_full[offsets[e]:offsets[e + 1]] = \
                core_out[el * C: el * C + gs[e]].astype(np.float32)
    return out_full, res.exec_time_ns


def kernel(hidden_states, gate_w, up_w, down_w, group_sizes):
    out, _ = _run(hidden_states, gate_w, up_w, down_w, group_sizes)
    return out

